# revision 70
# baseline (speedup 1.0000x reference)
"""AttentiveDensenet Trainium2 Bass kernel.

Data-parallel over batch B=8 across 8 NeuronCores (1 image per core).
Accepts full inputs, shards per-core on host, gathers full output.

Structure (per layer, per core; 1510us -> 561us -> ~485us over two
sessions):
  - All conv/KQV weights DMA'd in ONE batched transfer per (layer,
    tensor) from host-packed layouts. w1 (2.3MB) is double-buffered and
    its prefetch is GATED behind bn_h1(1) via a dummy read so the
    transfer never lands inside a BN collective window (ungated, its
    WAR dependency released exactly at stats time and the transfer
    inflated the collective 2-3x).
  - PE warmup matmuls at init ramp the p-state while input DMAs land;
    a fire-and-forget AllReduce warms the CC path so layer 0's first
    stats collective is cheap(er).
  - K/Q/V 1x1 convs as bf16 matmuls (x-tiles stationary) emitted in
    position-QUARTERS, interleaved with the attention quarters so DVE
    starts scoring quarter q while the PE still runs quarter q+1 KQV.
    Layers 0/1 (slack DVE): no bias matmuls; bias folds into the
    PSUM->SBUF copy as a DVE add vs a broadcast bias block. Layers 2/3
    (busy DVE): bias via a ones-row matmul, copies on Act.
  - Attention on DVE in bf16: packed products, single grouped f32
    reduce, f32 softmax; top-k (T=5) works on UNNORMALIZED exp via
    exact algebra (threshold e4+Z*eps, renorm sum+Z*eps), with a
    running (min,2nd-min) selection chain; final normalize writes
    bf16 attnb directly. Weighted sum with v/o d-major (col =
    pb*512+d*8+h) so the attn broadcast is packed -> DVE 2x mode.
  - o transposed pos->channel-major on the PE (is_transpose matmuls);
    conv1 weights host-permuted to absorb the d-major order. conv1 in
    4 row-chunks (10/9/8/5 rows - small last chunk so stats(0) follows
    quarter 3 quickly) pipelined behind the attention quarters;
    transposes are emitted before chunks so they are never delayed.
  - BN stats (sum/sumsq) AllReduce'd (2x 1KB per layer, contiguous
    [P,2] readback). Per-layer split of co=1 conv1 chunks between the
    quarter phase (feeds the PE during long late-layer attention) and
    the post-stats(0) phase (covers AllReduce(0) latency); AR(1) hides
    under conv2 ci=0 taps (both-co ci-split accumulation). ob1 cancels
    in training-mode BN and is dropped. h1 = relu(A*y1+B) on Act, split
    in two so conv2's first row-chunk starts early.
  - conv3x3 #2 + residual x += gamma*(h2 + ob2) via scalar_tensor_tensor;
    final layer streams the output DMA per resid chunk.
"""
import numpy as np
import ml_dtypes

import concourse.bacc as bacc
import concourse.mybir as mybir
import concourse.tile as tile
from concourse import bass_utils

L, C, B, H, W = 4, 256, 8, 32, 32
NH, KD = 8, 64
KH = NH * KD          # 512
HW = H * W            # 1024
P = 128
NC = 8                # cores
TOPK = 4
EPS = 1e-7
BN_EPS = 1e-5
PW = W + 2            # 34
PHW = PW * (H + 2)    # 1156
CHUNKS = [(0, 15), (15, 15), (30, 2)]
# conv1 row-chunks: chunk k consumable after transpose-quarter k; the last
# chunk is kept small so the stats(0) trigger follows quarter 3 quickly
CHUNKS4 = [(0, 10), (10, 9), (19, 8), (27, 5)]

f32 = mybir.dt.float32
bf16 = mybir.dt.bfloat16
AX = mybir.AxisListType
OP = mybir.AluOpType
ACTF = mybir.ActivationFunctionType

_compiled = {}


def _build(ncores=NC, layers=L):
    nc = bacc.Bacc(None, target_bir_lowering=False, debug=False, num_devices=ncores)

    xin = nc.dram_tensor("xin", [C, HW], f32, kind="ExternalInput").ap()
    wkqvd = nc.dram_tensor("wkqvd", [L, P, 3 * 2 * KH], bf16, kind="ExternalInput").ap()
    bkd = nc.dram_tensor("bkd", [L, 1, 3 * KH], bf16, kind="ExternalInput").ap()
    w1d = nc.dram_tensor("w1d", [L, P, 72 * P], bf16, kind="ExternalInput").ap()
    w2d = nc.dram_tensor("w2d", [L, P, 36 * P], bf16, kind="ExternalInput").ap()
    cstd = nc.dram_tensor("cstd", [L, P, 8], f32, kind="ExternalInput").ap()
    identd = nc.dram_tensor("identd", [P, P], bf16, kind="ExternalInput").ap()
    out = nc.dram_tensor("out", [C, HW], f32, kind="ExternalOutput").ap()

    with tile.TileContext(nc) as tc, \
         nc.allow_low_precision(reason="bf16 attention validated vs reference"):
        with tc.tile_pool(name="main", bufs=1) as mp, \
             tc.tile_pool(name="prodp", bufs=4) as prodp, \
             tc.tile_pool(name="psp", bufs=6, space="PSUM") as psp, \
             tc.tile_pool(name="dramp", bufs=4, space="DRAM") as dramp:

            # ---- persistent tiles ----
            x = [mp.tile([P, HW], f32, name=f"x{i}") for i in range(2)]
            xb = [mp.tile([P, HW], bf16, name=f"xb{i}") for i in range(2)]
            qbt = mp.tile([P, 8 * KH], bf16, name="qbt")
            kbt = [mp.tile([P, 8 * KH], bf16, name=f"kbt{i}") for i in range(L)]
            vbt = [mp.tile([P, 8 * KH], bf16, name=f"vbt{i}") for i in range(L)]
            obf = mp.tile([P, 8 * KH], bf16, name="obf")
            S = mp.tile([P, 5 * 64], f32, name="S")
            attnb = mp.tile([P, 5 * 64], bf16, name="attnb")
            attn = mp.tile([P, 5 * 64], f32, name="attn")
            mx = mp.tile([P, 64], f32, name="mx")
            zs = mp.tile([P, 64], f32, name="zs")
            dmin = mp.tile([P, 64], f32, name="dmin")
            mxp = mp.tile([P, 64], f32, name="mxp")
            opad = [mp.tile([P, PHW + 2], bf16, name=f"opad{i}") for i in range(4)]
            y1 = [mp.tile([P, HW], f32, name=f"y1_{i}") for i in range(2)]
            sqs = mp.tile([P, HW], f32, name="sqs")
            h1p = [mp.tile([P, PHW + 34], bf16, name=f"h1p{i}") for i in range(2)]
            st = mp.tile([P, 4], f32, name="st")
            gred = mp.tile([P, 4], f32, name="gred")
            pre_s = mp.tile([1, 8], f32, name="pre_s")
            ones1 = mp.tile([1, P], bf16, name="ones1")
            ident = mp.tile([P, P], bf16, name="ident")
            wscr = mp.tile([P, 512], bf16, name="wscr")
            wgate = mp.tile([1, 1], f32, name="wgate")
            bb = mp.tile([P, 3 * KH], bf16, name="bb")
            # weight buffers. w1 (the 2.3MB one) is double-buffered and
            # prefetched at layer start so its DMA never lands inside the
            # BN collective window; the rest reload for layer l+1 right
            # after their last layer-l consumer, hidden under compute.
            wkv = mp.tile([P, 3 * 2 * KH], bf16, name="wkv")
            bks = mp.tile([1, 3 * KH], bf16, name="bks")
            w1s = [mp.tile([P, 72 * P], bf16, name=f"w1s{i}") for i in range(2)]
            w2s = mp.tile([P, 36 * P], bf16, name="w2s")
            cst = mp.tile([P, 8], f32, name="cst")
            # BN scratch
            t1 = [mp.tile([P, 1], f32, name=f"t1_{i}") for i in range(2)]
            Ac = [mp.tile([P, 1], f32, name=f"Ac{i}") for i in range(2)]
            Bc = [mp.tile([P, 1], f32, name=f"Bc{i}") for i in range(2)]
            sq = [mp.tile([P, 1], f32, name=f"sq{i}") for i in range(2)]
            vart = [mp.tile([P, 1], f32, name=f"vart{i}") for i in range(2)]
            stdt = [mp.tile([P, 1], f32, name=f"stdt{i}") for i in range(2)]

            def copy_on(e, dst, src):
                if e is nc.scalar:
                    nc.scalar.copy(dst, src)
                else:
                    e.tensor_copy(dst, src)

            def load_kqv_weights(l):
                nc.sync.dma_start(wkv[:], wkqvd[l])
                nc.sync.dma_start(bks[:], bkd[l])

            # ---- init ----
            # PE warmup: dummy matmuls ramp the PE p-state to 8/8 while the
            # input/weight DMAs land, so layer 0 runs at full clock.
            nc.vector.memset(wscr[:], 0.5)
            for i in range(28):
                wps = psp.tile([P, 512], f32, name="wps", tag="ps")
                nc.tensor.matmul(wps[:], wscr[:, 0:P], wscr[:],
                                 start=True, stop=True)
            # fire-and-forget warmup collective: pays the CC cold-start cost
            # concurrently with layer-0 compute so layer 0's first stats
            # collective rides a warm CC path (nothing is gated on it).
            nc.vector.memset(pre_s[:], 1.0)
            pre_i = dramp.tile([1, 8], f32, name="pre_i")
            pre_o = dramp.tile([1, 8], f32, name="pre_o", addr_space="Shared")
            nc.sync.dma_start(pre_i[0], pre_s[0])
            nc.gpsimd.collective_compute(
                "AllReduce", OP.add, replica_groups=[list(range(ncores))],
                ins=[pre_i.opt()], outs=[pre_o.opt()])
            for i in range(2):
                nc.sync.dma_start(x[i][:], xin[i * P:(i + 1) * P, :])
                nc.scalar.copy(xb[i][:], x[i][:])
            nc.sync.dma_start(ident[:], identd)
            for i in range(4):
                nc.vector.memset(opad[i][:], 0)
            for i in range(2):
                nc.vector.memset(h1p[i][:], 0)
            nc.vector.memset(ones1[:], 1.0)
            load_kqv_weights(0)
            nc.sync.dma_start(w1s[0][:], w1d[0])
            nc.sync.dma_start(w2s[:], w2d[0])
            nc.sync.dma_start(cst[:], cstd[0])

            S3g = S[:].rearrange("p (t g) -> p g t", g=64)
            S3t = S[:].rearrange("p (t g) -> p t g", g=64)
            ab3g = attn[:].rearrange("p (t g) -> p g t", g=64)
            ab3t = attn[:].rearrange("p (t g) -> p t g", g=64)
            abb3t = attnb[:].rearrange("p (t g) -> p t g", g=64)
            abb3g = attnb[:].rearrange("p (t g) -> p g t", g=64)

            for l in range(layers):
                R, T = l + 1, l + 2
                w1c = w1s[l % 2]
                bng = [cst[:, 4 * co + 0:4 * co + 1] for co in range(2)]
                bnb = [cst[:, 4 * co + 1:4 * co + 2] for co in range(2)]
                gob2 = [cst[:, 4 * co + 2:4 * co + 3] for co in range(2)]
                gam = [cst[:, 4 * co + 3:4 * co + 4] for co in range(2)]

                # ---- KQV (quarters; PE streams while copies drain) ----
                # layers 0/1 have DVE slack: drop the 24 per-(c,pb) bias
                # matmuls (5.1us PE each layer) and fold the bias into the
                # PSUM->SBUF copy as a DVE add against a broadcast bias block
                # (built once per layer with 3 matmuls). Layers 2/3 keep the
                # PE bias path so the busier DVE attention chain stays clean.
                dve_bias = l < 2
                if dve_bias:
                    for c in range(3):
                        bps = psp.tile([P, KH], f32, name="ps", tag="ps")
                        nc.tensor.matmul(bps[:], ones1[:],
                                         bks[0:1, c * KH:(c + 1) * KH],
                                         start=True, stop=True)
                        nc.scalar.copy(bb[:, c * KH:(c + 1) * KH], bps[:])

                def kqv_quarter(qb_, cnt=[0]):
                    for c, dest in ((0, kbt[l]), (1, vbt[l]), (2, qbt)):
                        for pbh in range(2):
                            pb = qb_ * 2 + pbh
                            ps = psp.tile([P, KH], f32, name="ps", tag="ps")
                            if not dve_bias:
                                nc.tensor.matmul(ps[:], ones1[:],
                                                 bks[0:1, c * KH:(c + 1) * KH],
                                                 start=True, stop=False)
                            nc.tensor.matmul(ps[:], xb[0][:, pb * P:(pb + 1) * P],
                                             wkv[:, (2 * c + 0) * KH:(2 * c + 1) * KH],
                                             start=dve_bias, stop=False)
                            nc.tensor.matmul(ps[:], xb[1][:, pb * P:(pb + 1) * P],
                                             wkv[:, (2 * c + 1) * KH:(2 * c + 2) * KH],
                                             start=False, stop=True)
                            if dve_bias:
                                nc.vector.tensor_tensor(
                                    dest[:, pb * KH:(pb + 1) * KH], ps[:],
                                    bb[:, c * KH:(c + 1) * KH], OP.add)
                            else:
                                e = (nc.scalar, nc.vector)[cnt[0] % 2] \
                                    if qb_ == 0 else nc.scalar
                                copy_on(e, dest[:, pb * KH:(pb + 1) * KH], ps[:])
                            cnt[0] += 1

                def conv1_part(parts):
                    for co, ck in parts:
                        i0, nr = CHUNKS4[ck]
                        nw = PW * nr
                        ps = psp.tile([P, 512], f32, name="ps", tag="ps")
                        for tap in range(9):
                            ty, tx = divmod(tap, 3)
                            base = PW * (i0 + ty) + tx
                            for ci in range(4):
                                off = ((co * 9 + tap) * 4 + ci) * P
                                nc.tensor.matmul(ps[:, 0:nw], w1c[:, off:off + P],
                                                 opad[ci][:, base:base + nw],
                                                 start=(tap == 0 and ci == 0),
                                                 stop=(tap == 8 and ci == 3))
                        copy_on(nc.vector if ck % 2 else nc.scalar,
                                y1[co][:, W * i0:W * (i0 + nr)].rearrange(
                                    "c (i j) -> c i j", j=W),
                                ps[:, 0:nw].rearrange("c (i j) -> c i j", j=PW)[:, :, 0:W])

                # ---- attention quarters (DVE/Act), interleaved with KQV ----
                def attn_quarter(hb):
                    CL = slice(hb * 1024, (hb + 1) * 1024)
                    GS = slice(hb * 16, hb * 16 + 16)
                    # scores
                    for t in range(R):
                        pr = prodp.tile([P, 2048], bf16, name="prod", tag="pr")
                        nc.vector.tensor_mul(pr[:, 0:1024], qbt[:, CL],
                                             kbt[t][:, CL])
                        nc.vector.tensor_reduce(
                            out=S3t[:, t, GS],
                            in_=pr[:, 0:1024].rearrange("p (g d) -> p g d", d=KD),
                            axis=AX.X, op=OP.add)
                    nc.vector.memset(S3t[:, R, GS], 0)  # zero-key slot
                    # softmax over T slots
                    nc.vector.tensor_reduce(out=mx[:, GS], in_=S3g[:, GS, 0:T],
                                            axis=AX.X, op=OP.max)
                    nc.vector.tensor_tensor(
                        ab3g[:, GS, 0:T], S3g[:, GS, 0:T],
                        mx[:, GS].unsqueeze(2).broadcast_to([P, 16, T]), OP.subtract)
                    nc.scalar.activation(ab3t[:, 0:T, GS], ab3t[:, 0:T, GS], ACTF.Exp)
                    nc.vector.tensor_reduce(out=zs[:, GS], in_=ab3g[:, GS, 0:T],
                                            axis=AX.X, op=OP.add)
                    if T > TOPK:
                        # sparse top-k on UNNORMALIZED e (exact algebra:
                        # w = max(e/Z - e4/Z - EPS, 0), out = w/(sum w + EPS)
                        # == max(e - e4 - Z*EPS, 0)/(sum(...) + Z*EPS)).
                        # 2nd-smallest of e via running (min, 2nd-min) chain.
                        nc.vector.tensor_tensor(mxp[:, GS], ab3t[:, 0, GS],
                                                ab3t[:, 1, GS], OP.min)
                        nc.vector.tensor_tensor(dmin[:, GS], ab3t[:, 0, GS],
                                                ab3t[:, 1, GS], OP.max)
                        for i in range(2, T):
                            nc.vector.tensor_tensor(mx[:, GS], mxp[:, GS],
                                                    ab3t[:, i, GS], OP.max)
                            nc.vector.tensor_tensor(dmin[:, GS], dmin[:, GS],
                                                    mx[:, GS], OP.min)
                            if i + 1 < T:
                                nc.vector.tensor_tensor(mxp[:, GS], mxp[:, GS],
                                                        ab3t[:, i, GS], OP.min)
                        # threshold = e_(4) + Z*EPS
                        nc.vector.scalar_tensor_tensor(
                            out=dmin[:, GS], in0=zs[:, GS], scalar=EPS,
                            in1=dmin[:, GS], op0=OP.mult, op1=OP.add)
                        nc.vector.tensor_tensor(
                            ab3g[:, GS, 0:T], ab3g[:, GS, 0:T],
                            dmin[:, GS].unsqueeze(2).broadcast_to([P, 16, T]),
                            OP.subtract)
                        nc.vector.tensor_scalar_max(ab3g[:, GS, 0:T],
                                                    ab3g[:, GS, 0:T], 0.0)
                        nc.vector.tensor_reduce(out=mx[:, GS],
                                                in_=ab3g[:, GS, 0:T],
                                                axis=AX.X, op=OP.add)
                        nc.vector.scalar_tensor_tensor(
                            out=mx[:, GS], in0=zs[:, GS], scalar=EPS,
                            in1=mx[:, GS], op0=OP.mult, op1=OP.add)
                        nc.vector.reciprocal(mx[:, GS], mx[:, GS])
                        nc.vector.tensor_tensor(
                            abb3g[:, GS, 0:T], ab3g[:, GS, 0:T],
                            mx[:, GS].unsqueeze(2).broadcast_to([P, 16, T]),
                            OP.mult)
                    else:
                        nc.vector.reciprocal(zs[:, GS], zs[:, GS])
                        nc.vector.tensor_tensor(
                            abb3g[:, GS, 0:T], ab3g[:, GS, 0:T],
                            zs[:, GS].unsqueeze(2).broadcast_to([P, 16, T]),
                            OP.mult)
                    # weighted sum -> obf half. v/obf are d-major
                    # (col = pb*512 + d*8 + h) so the attn broadcast is
                    # packed in the last dim -> DVE 2x mode.
                    for t in range(R):
                        abb = abb3t[:, t, GS].rearrange(
                            "p (b h) -> p b h", h=8).unsqueeze(2).broadcast_to(
                            [P, 2, KD, 8])
                        vv = vbt[t][:, CL].rearrange(
                            "p (b d h) -> p b d h", d=KD, h=8)
                        if t == 0:
                            nc.vector.tensor_tensor(
                                obf[:, CL].rearrange("p (b d h) -> p b d h",
                                                     d=KD, h=8),
                                vv, abb, OP.mult)
                        else:
                            tm = prodp.tile([P, 2048], bf16, name="wtm", tag="pr")
                            nc.vector.tensor_tensor(
                                tm[:, 0:1024].rearrange("p (b d h) -> p b d h",
                                                        d=KD, h=8),
                                vv, abb, OP.mult)
                            nc.vector.tensor_add(obf[:, CL], obf[:, CL],
                                                 tm[:, 0:1024])
                def transp_quarter(hb):
                    # PE transpose of this quarter into opad
                    for q in range(4):
                        tp = psp.tile([P, 512], bf16, name="tp", tag="ps")
                        for pbh in range(2):
                            pb = hb * 2 + pbh
                            nc.tensor.matmul(
                                tp[:, pbh * P:(pbh + 1) * P],
                                obf[:, pb * KH + q * P: pb * KH + (q + 1) * P],
                                ident[:], is_transpose=True, skip_group_check=True)
                        opv = opad[q][:, 0:PHW].rearrange("c (i j) -> c i j", j=PW)
                        copy_on(nc.scalar,
                                opv[:, 1 + 8 * hb:9 + 8 * hb, 1:W + 1],
                                tp[:, 0:256].rearrange("c (i j) -> c i j", j=W))

                # interleave: kqv(q) then attn(q) so DVE starts attention at
                # quarter granularity; transposes + conv1 mini-chunks trail.
                # conv1 mini-chunk k needs opad rows <= 8(k+1) = quarters
                # 0..k; co=0 runs ahead of co=1 so stats(0)+AR(0) issue early.
                # Per-layer split of co=1 chunks: late layers have a long DVE
                # attention chain, so embed more chunks to keep the PE fed;
                # early layers keep chunks back as AR(0) latency cover.
                embed = {0: [], 1: [], 2: [(1, 0)],
                         3: [(1, 0), (1, 1)]}[l]
                cover = [c for c in [(1, 0), (1, 1), (1, 2), (1, 3)]
                         if c not in embed]
                for q_ in range(4):
                    kqv_quarter(q_)
                    attn_quarter(q_)
                if l + 1 < layers:
                    load_kqv_weights(l + 1)
                for q_ in range(4):
                    # transposes first in the PE queue: conv1 chunks must
                    # never delay the transpose feeding the next chunk
                    transp_quarter(q_)
                    if q_ == 1:
                        conv1_part([(0, 0)])
                    elif q_ == 2:
                        conv1_part([(0, 1)] + embed[:1])
                    elif q_ == 3:
                        conv1_part([(0, 2)] + embed[1:])

                conv1_part([(0, 3)])

                # ---- stats + AllReduce per channel-half, pipelined ----
                def stats_send(i):
                    nc.vector.tensor_reduce(out=st[:, 2 * i:2 * i + 1], in_=y1[i][:],
                                            axis=AX.X, op=OP.add)
                    nc.scalar.square(sqs[:], y1[i][:])
                    nc.vector.tensor_reduce(out=st[:, 2 * i + 1:2 * i + 2],
                                            in_=sqs[:], axis=AX.X, op=OP.add)
                    cci = dramp.tile([1, 2 * P], f32, name="cci")
                    cco = dramp.tile([1, 2 * P], f32, name="cco",
                                     addr_space="Shared")
                    nc.sync.dma_start(cci[0].rearrange("(p j) -> p j", j=2),
                                      st[:, 2 * i:2 * i + 2])
                    nc.gpsimd.collective_compute(
                        "AllReduce", OP.add,
                        replica_groups=[list(range(ncores))],
                        ins=[cci.opt()], outs=[cco.opt()])
                    nc.sync.dma_start(
                        gred[:, 2 * i:2 * i + 2],
                        cco[0].rearrange("(p j) -> p j", j=2))

                stats_send(0)
                conv1_part(cover)
                stats_send(1)
                for co in range(2):
                    nc.scalar.add(x[co][:], x[co][:], gob2[co])

                # ---- BN coef + h1 + conv2 (ci-split overlaps AG latency) ----
                NTOT = float(ncores * HW)

                def bn_h1(i):
                    nc.vector.tensor_scalar_mul(t1[i][:], gred[:, 2 * i:2 * i + 1],
                                                1.0 / NTOT)
                    nc.vector.tensor_scalar_mul(vart[i][:],
                                                gred[:, 2 * i + 1:2 * i + 2],
                                                1.0 / NTOT)
                    nc.vector.tensor_mul(sq[i][:], t1[i][:], t1[i][:])
                    nc.vector.tensor_sub(vart[i][:], vart[i][:], sq[i][:])
                    nc.vector.tensor_scalar_add(vart[i][:], vart[i][:], BN_EPS)
                    nc.scalar.activation(stdt[i][:], vart[i][:], ACTF.Sqrt)
                    nc.vector.reciprocal(stdt[i][:], stdt[i][:])
                    nc.vector.tensor_mul(Ac[i][:], bng[i], stdt[i][:])
                    nc.vector.tensor_mul(sq[i][:], t1[i][:], Ac[i][:])
                    nc.vector.tensor_sub(Bc[i][:], bnb[i], sq[i][:])
                    # split the relu so conv2's first row-chunk (reads h1
                    # rows <= 15) can start before the full h1 is written
                    h1v = h1p[i][:, 0:PHW].rearrange("c (i j) -> c i j", j=PW)
                    y1v = y1[i][:].rearrange("c (i j) -> c i j", j=W)
                    nc.scalar.activation(
                        h1v[:, 1:17, 1:W + 1], y1v[:, 0:16, :],
                        ACTF.Relu, bias=Bc[i][:], scale=Ac[i][:])
                    nc.scalar.activation(
                        h1v[:, 17:H + 1, 1:W + 1], y1v[:, 16:H, :],
                        ACTF.Relu, bias=Bc[i][:], scale=Ac[i][:])

                def conv2_taps(ps2, co, ci, start, stop):
                    for ck, (i0, nr) in enumerate(CHUNKS):
                        nw = PW * nr
                        for tap in range(9):
                            ty, tx = divmod(tap, 3)
                            base = PW * (i0 + ty) + tx
                            off = ((co * 9 + tap) * 2 + ci) * P
                            nc.tensor.matmul(
                                ps2[ck][:, 0:nw], w2s[:, off:off + P],
                                h1p[ci][:, base:base + nw],
                                start=(start and tap == 0),
                                stop=(stop and tap == 8))

                def resid(ps2, co):
                    for ck, (i0, nr) in enumerate(CHUNKS):
                        nw = PW * nr
                        xsl = x[co][:, W * i0:W * (i0 + nr)].rearrange(
                            "c (i j) -> c i j", j=W)
                        nc.vector.scalar_tensor_tensor(
                            out=xsl,
                            in0=ps2[ck][:, 0:nw].rearrange(
                                "c (i j) -> c i j", j=PW)[:, :, 0:W],
                            scalar=gam[co], in1=xsl, op0=OP.mult, op1=OP.add)
                        if l == layers - 1:
                            # final layer: stream the output per chunk so the
                            # last DMA overlaps the remaining resid work
                            nc.sync.dma_start(
                                out[co * P:(co + 1) * P,
                                    W * i0:W * (i0 + nr)],
                                x[co][:, W * i0:W * (i0 + nr)])
                    if l < layers - 1:
                        # split the bf16 copy: next layer's kqv quarter 0
                        # needs only the first 256 positions, so release
                        # that slice first
                        nc.scalar.copy(xb[co][:, 0:2 * P], x[co][:, 0:2 * P])
                        nc.scalar.copy(xb[co][:, 2 * P:], x[co][:, 2 * P:])

                # ci-split: both co groups' ci=0 taps hide AG(1) latency
                bn_h1(0)
                ps20 = [psp.tile([P, 512], f32, name="ps", tag="ps")
                        for _ in range(3)]
                ps21 = [psp.tile([P, 512], f32, name="ps", tag="ps")
                        for _ in range(3)]
                conv2_taps(ps20, 0, 0, True, False)
                conv2_taps(ps21, 1, 0, True, False)
                bn_h1(1)
                if l + 1 < layers:
                    # gate the 2.3MB w1 prefetch behind bn_h1(1): the dummy
                    # read adds a WAR dep so the transfer fires only after
                    # both BN collectives have completed, never inside them.
                    # (must read an INTERIOR h1p element — bn_h1 writes only
                    # rows/cols 1.., the [0,0] padding never changes)
                    nc.vector.tensor_tensor(wgate[:], w1s[(l + 1) % 2][0:1, 0:1],
                                            h1p[1][0:1, PW + 1:PW + 2], OP.add)
                    nc.sync.dma_start(w1s[(l + 1) % 2][:], w1d[l + 1])
                conv2_taps(ps20, 0, 1, False, True)
                resid(ps20, 0)
                conv2_taps(ps21, 1, 1, False, True)
                if l + 1 < layers:
                    nc.sync.dma_start(w2s[:], w2d[l + 1])
                resid(ps21, 1)
                if l + 1 < layers:
                    nc.sync.dma_start(cst[:], cstd[l + 1])

    nc.compile()
    return nc


def _host_prep(inputs):
    bf = ml_dtypes.bfloat16
    kw, kb = inputs["kw"], inputs["kb"]
    qw, qb = inputs["qw"], inputs["qb"]
    vw, vb = inputs["vw"], inputs["vb"]
    ow1, ow2 = inputs["ow1"], inputs["ow2"]
    ob2, gammas = inputs["ob2"], inputs["gammas"]

    def packkqv(w):  # [L, KH, C] -> [L, P, 2, KH]
        return w.reshape(L, KH, 2, P).transpose(0, 3, 2, 1)

    # v (and o) use d-major channel order: col d*8+h holds row h*64+d.
    # This makes the attention weighted-sum broadcast packed for DVE 2x.
    permdh = np.array([(c % 8) * 64 + c // 8 for c in range(KH)])
    d = {}
    wk3 = np.stack([packkqv(kw), packkqv(vw)[..., permdh],
                    packkqv(qw / 8.0)], axis=2)
    d["wkqvd"] = np.ascontiguousarray(wk3.reshape(L, P, 3 * 2 * KH)).astype(bf)
    bk3 = np.stack([kb, vb[:, permdh], qb / 8.0], axis=1)
    d["bkd"] = np.ascontiguousarray(bk3.reshape(L, 1, 3 * KH)).astype(bf)
    # conv1 input channels arrive via the PE transpose of d-major o:
    # opad[k] partition j holds channel c = (j%8)*64 + 16k + j//8.
    cinidx = np.array([[(j % 8) * 64 + 16 * k + j // 8 for j in range(P)]
                       for k in range(4)])
    a1 = ow1[:, :, cinidx]                      # [L, 256, 4, 128, 3, 3]
    a1 = a1.reshape(L, 2, P, 4, P, 3, 3).transpose(0, 4, 1, 5, 6, 3, 2)
    d["w1d"] = np.ascontiguousarray(a1.reshape(L, P, 72 * P)).astype(bf)
    a2 = ow2.reshape(L, 2, P, 2, P, 3, 3).transpose(0, 4, 1, 5, 6, 3, 2)
    d["w2d"] = np.ascontiguousarray(a2.reshape(L, P, 36 * P)).astype(bf)
    cstv = np.zeros((L, 2, P, 4), np.float32)
    cstv[..., 0] = inputs["bn_g"].reshape(L, 2, P)
    cstv[..., 1] = inputs["bn_b"].reshape(L, 2, P)
    cstv[..., 2] = (gammas[:, None] * ob2).reshape(L, 2, P)
    cstv[..., 3] = gammas[:, None, None]
    d["cstd"] = np.ascontiguousarray(
        cstv.transpose(0, 2, 1, 3).reshape(L, P, 8)).astype(np.float32)
    d["identd"] = np.eye(P, dtype=np.float32).astype(bf)
    return d


def kernel(**inputs):
    if "nc" not in _compiled:
        _compiled["nc"] = _build()
    nc = _compiled["nc"]
    shared = _host_prep(inputs)
    x = np.ascontiguousarray(inputs["x"].reshape(B, C, HW)).astype(np.float32)
    in_maps = []
    for c in range(NC):
        m = dict(shared)
        m["xin"] = x[c]
        in_maps.append(m)
    res = bass_utils.run_bass_kernel_spmd(nc, in_maps, core_ids=list(range(NC)))
    outs = np.stack([res.results[c]["out"] for c in range(NC)])
    return outs.reshape(B, C, H, W).astype(np.float32)



# revision 72
# speedup vs baseline: 1.0978x; 1.0978x over previous
"""AttentiveDensenet Trainium2 Bass kernel.

Data-parallel over batch B=8 across 8 NeuronCores (1 image per core).
Accepts full inputs, shards per-core on host, gathers full output.

Structure (per layer, per core; 1510us -> 561us -> ~485us over two
sessions):
  - All conv/KQV weights DMA'd in ONE batched transfer per (layer,
    tensor) from host-packed layouts. w1 (2.3MB) is double-buffered and
    its prefetch is GATED behind bn_h1(1) via a dummy read so the
    transfer never lands inside a BN collective window (ungated, its
    WAR dependency released exactly at stats time and the transfer
    inflated the collective 2-3x).
  - PE warmup matmuls at init ramp the p-state while input DMAs land;
    a fire-and-forget AllReduce warms the CC path so layer 0's first
    stats collective is cheap(er).
  - K/Q/V 1x1 convs as bf16 matmuls (x-tiles stationary) emitted in
    position-QUARTERS, interleaved with the attention quarters so DVE
    starts scoring quarter q while the PE still runs quarter q+1 KQV.
    Layers 0/1 (slack DVE): no bias matmuls; bias folds into the
    PSUM->SBUF copy as a DVE add vs a broadcast bias block. Layers 2/3
    (busy DVE): bias via a ones-row matmul, copies on Act.
  - Attention on DVE in bf16: packed products, single grouped f32
    reduce, f32 softmax; top-k (T=5) works on UNNORMALIZED exp via
    exact algebra (threshold e4+Z*eps, renorm sum+Z*eps), with a
    running (min,2nd-min) selection chain; final normalize writes
    bf16 attnb directly. Weighted sum with v/o d-major (col =
    pb*512+d*8+h) so the attn broadcast is packed -> DVE 2x mode.
  - o transposed pos->channel-major on the PE (is_transpose matmuls);
    conv1 weights host-permuted to absorb the d-major order. conv1 in
    4 row-chunks (10/9/8/5 rows - small last chunk so stats(0) follows
    quarter 3 quickly) pipelined behind the attention quarters;
    transposes are emitted before chunks so they are never delayed.
  - BN stats (sum/sumsq) AllReduce'd (2x 1KB per layer, contiguous
    [P,2] readback). Per-layer split of co=1 conv1 chunks between the
    quarter phase (feeds the PE during long late-layer attention) and
    the post-stats(0) phase (covers AllReduce(0) latency); AR(1) hides
    under conv2 ci=0 taps (both-co ci-split accumulation). ob1 cancels
    in training-mode BN and is dropped. h1 = relu(A*y1+B) on Act, split
    in two so conv2's first row-chunk starts early.
  - conv3x3 #2 + residual x += gamma*(h2 + ob2) via scalar_tensor_tensor;
    final layer streams the output DMA per resid chunk.
"""
import numpy as np
import ml_dtypes

import concourse.bacc as bacc
import concourse.mybir as mybir
import concourse.tile as tile
from concourse import bass_utils

L, C, B, H, W = 4, 256, 8, 32, 32
NH, KD = 8, 64
KH = NH * KD          # 512
HW = H * W            # 1024
P = 128
NC = 8                # cores
TOPK = 4
EPS = 1e-7
BN_EPS = 1e-5
PW = W + 2            # 34
PHW = PW * (H + 2)    # 1156
CHUNKS = [(0, 15), (15, 15), (30, 2)]
# conv1 row-chunks: chunk k consumable after transpose-quarter k; the last
# chunk is kept small so the stats(0) trigger follows quarter 3 quickly
CHUNKS4 = [(0, 10), (10, 9), (19, 8), (27, 5)]

f32 = mybir.dt.float32
bf16 = mybir.dt.bfloat16
AX = mybir.AxisListType
OP = mybir.AluOpType
ACTF = mybir.ActivationFunctionType

_compiled = {}


def _build(ncores=NC, layers=L):
    nc = bacc.Bacc(None, target_bir_lowering=False, debug=False, num_devices=ncores)

    xin = nc.dram_tensor("xin", [C, HW], f32, kind="ExternalInput").ap()
    wkqvd = nc.dram_tensor("wkqvd", [L, P, 3 * 2 * KH], bf16, kind="ExternalInput").ap()
    bkd = nc.dram_tensor("bkd", [L, 1, 3 * KH], bf16, kind="ExternalInput").ap()
    w1d = nc.dram_tensor("w1d", [L, P, 72 * P], bf16, kind="ExternalInput").ap()
    w2d = nc.dram_tensor("w2d", [L, P, 36 * P], bf16, kind="ExternalInput").ap()
    cstd = nc.dram_tensor("cstd", [L, P, 8], f32, kind="ExternalInput").ap()
    identd = nc.dram_tensor("identd", [P, P], bf16, kind="ExternalInput").ap()
    out = nc.dram_tensor("out", [C, HW], f32, kind="ExternalOutput").ap()

    with tile.TileContext(nc) as tc, \
         nc.allow_low_precision(reason="bf16 attention validated vs reference"):
        with tc.tile_pool(name="main", bufs=1) as mp, \
             tc.tile_pool(name="prodp", bufs=4) as prodp, \
             tc.tile_pool(name="psp", bufs=6, space="PSUM") as psp, \
             tc.tile_pool(name="dramp", bufs=4, space="DRAM") as dramp:

            # ---- persistent tiles ----
            x = [mp.tile([P, HW], f32, name=f"x{i}") for i in range(2)]
            xb = [mp.tile([P, HW], bf16, name=f"xb{i}") for i in range(2)]
            qbt = mp.tile([P, 8 * KH], bf16, name="qbt")
            kbt = [mp.tile([P, 8 * KH], bf16, name=f"kbt{i}") for i in range(L)]
            vbt = [mp.tile([P, 8 * KH], bf16, name=f"vbt{i}") for i in range(L)]
            obf = mp.tile([P, 8 * KH], bf16, name="obf")
            S = mp.tile([P, 5 * 64], f32, name="S")
            attnb = mp.tile([P, 5 * 64], bf16, name="attnb")
            attn = mp.tile([P, 5 * 64], f32, name="attn")
            mx = mp.tile([P, 64], f32, name="mx")
            zs = mp.tile([P, 64], f32, name="zs")
            dmin = mp.tile([P, 64], f32, name="dmin")
            mxp = mp.tile([P, 64], f32, name="mxp")
            opad = [mp.tile([P, PHW + 2], bf16, name=f"opad{i}") for i in range(4)]
            y1 = [mp.tile([P, HW], f32, name=f"y1_{i}") for i in range(2)]
            sqs = mp.tile([P, HW], f32, name="sqs")
            h1p = [mp.tile([P, PHW + 34], bf16, name=f"h1p{i}") for i in range(2)]
            st = mp.tile([P, 4], f32, name="st")
            gred = mp.tile([P, 4], f32, name="gred")
            pre_s = mp.tile([1, 8], f32, name="pre_s")
            ones1 = mp.tile([1, P], bf16, name="ones1")
            ident = mp.tile([P, P], bf16, name="ident")
            wscr = mp.tile([P, 512], bf16, name="wscr")
            wgate = mp.tile([1, 1], f32, name="wgate")
            bb = mp.tile([P, 3 * KH], bf16, name="bb")
            # weight buffers. w1 (the 2.3MB one) is double-buffered and
            # prefetched at layer start so its DMA never lands inside the
            # BN collective window; the rest reload for layer l+1 right
            # after their last layer-l consumer, hidden under compute.
            wkv = mp.tile([P, 3 * 2 * KH], bf16, name="wkv")
            bks = mp.tile([1, 3 * KH], bf16, name="bks")
            w1s = [mp.tile([P, 72 * P], bf16, name=f"w1s{i}") for i in range(2)]
            w2s = mp.tile([P, 36 * P], bf16, name="w2s")
            cst = mp.tile([P, 8], f32, name="cst")
            # BN scratch
            t1 = [mp.tile([P, 1], f32, name=f"t1_{i}") for i in range(2)]
            Ac = [mp.tile([P, 1], f32, name=f"Ac{i}") for i in range(2)]
            Bc = [mp.tile([P, 1], f32, name=f"Bc{i}") for i in range(2)]
            sq = [mp.tile([P, 1], f32, name=f"sq{i}") for i in range(2)]
            vart = [mp.tile([P, 1], f32, name=f"vart{i}") for i in range(2)]
            stdt = [mp.tile([P, 1], f32, name=f"stdt{i}") for i in range(2)]

            def copy_on(e, dst, src):
                if e is nc.scalar:
                    nc.scalar.copy(dst, src)
                else:
                    e.tensor_copy(dst, src)

            def load_kqv_weights(l):
                nc.sync.dma_start(wkv[:], wkqvd[l])
                nc.sync.dma_start(bks[:], bkd[l])

            # ---- init ----
            # PE warmup: dummy matmuls ramp the PE p-state to 8/8 while the
            # input/weight DMAs land, so layer 0 runs at full clock.
            nc.vector.memset(wscr[:], 0.5)
            for i in range(28):
                wps = psp.tile([P, 512], f32, name="wps", tag="ps")
                nc.tensor.matmul(wps[:], wscr[:, 0:P], wscr[:],
                                 start=True, stop=True)
            # fire-and-forget warmup collective: pays the CC cold-start cost
            # concurrently with layer-0 compute so layer 0's first stats
            # collective rides a warm CC path (nothing is gated on it).
            nc.vector.memset(pre_s[:], 1.0)
            pre_i = dramp.tile([1, 8], f32, name="pre_i")
            pre_o = dramp.tile([1, 8], f32, name="pre_o", addr_space="Shared")
            nc.sync.dma_start(pre_i[0], pre_s[0])
            nc.gpsimd.collective_compute(
                "AllReduce", OP.add, replica_groups=[list(range(ncores))],
                ins=[pre_i.opt()], outs=[pre_o.opt()])
            for i in range(2):
                nc.sync.dma_start(x[i][:], xin[i * P:(i + 1) * P, :])
                nc.scalar.copy(xb[i][:], x[i][:])
            nc.sync.dma_start(ident[:], identd)
            for i in range(4):
                nc.vector.memset(opad[i][:], 0)
            for i in range(2):
                nc.vector.memset(h1p[i][:], 0)
            nc.vector.memset(ones1[:], 1.0)
            load_kqv_weights(0)
            nc.sync.dma_start(w1s[0][:], w1d[0])
            nc.sync.dma_start(w2s[:], w2d[0])
            nc.sync.dma_start(cst[:], cstd[0])

            S3g = S[:].rearrange("p (t g) -> p g t", g=64)
            S3t = S[:].rearrange("p (t g) -> p t g", g=64)
            ab3g = attn[:].rearrange("p (t g) -> p g t", g=64)
            ab3t = attn[:].rearrange("p (t g) -> p t g", g=64)
            abb3t = attnb[:].rearrange("p (t g) -> p t g", g=64)
            abb3g = attnb[:].rearrange("p (t g) -> p g t", g=64)

            for l in range(layers):
                R, T = l + 1, l + 2
                w1c = w1s[l % 2]
                bng = [cst[:, 4 * co + 0:4 * co + 1] for co in range(2)]
                bnb = [cst[:, 4 * co + 1:4 * co + 2] for co in range(2)]
                gob2 = [cst[:, 4 * co + 2:4 * co + 3] for co in range(2)]
                gam = [cst[:, 4 * co + 3:4 * co + 4] for co in range(2)]

                # ---- KQV (quarters; PE streams while copies drain) ----
                # layers 0/1 have DVE slack: drop the 24 per-(c,pb) bias
                # matmuls (5.1us PE each layer) and fold the bias into the
                # PSUM->SBUF copy as a DVE add against a broadcast bias block
                # (built once per layer with 3 matmuls). Layers 2/3 keep the
                # PE bias path so the busier DVE attention chain stays clean.
                dve_bias = l < 2
                if dve_bias:
                    for c in range(3):
                        bps = psp.tile([P, KH], f32, name="ps", tag="ps")
                        nc.tensor.matmul(bps[:], ones1[:],
                                         bks[0:1, c * KH:(c + 1) * KH],
                                         start=True, stop=True)
                        nc.scalar.copy(bb[:, c * KH:(c + 1) * KH], bps[:])

                def kqv_quarter(qb_, cnt=[0]):
                    for c, dest in ((0, kbt[l]), (1, vbt[l]), (2, qbt)):
                        for pbh in range(2):
                            pb = qb_ * 2 + pbh
                            ps = psp.tile([P, KH], f32, name="ps", tag="ps")
                            if not dve_bias:
                                nc.tensor.matmul(ps[:], ones1[:],
                                                 bks[0:1, c * KH:(c + 1) * KH],
                                                 start=True, stop=False)
                            nc.tensor.matmul(ps[:], xb[0][:, pb * P:(pb + 1) * P],
                                             wkv[:, (2 * c + 0) * KH:(2 * c + 1) * KH],
                                             start=dve_bias, stop=False)
                            nc.tensor.matmul(ps[:], xb[1][:, pb * P:(pb + 1) * P],
                                             wkv[:, (2 * c + 1) * KH:(2 * c + 2) * KH],
                                             start=False, stop=True)
                            if dve_bias:
                                nc.vector.tensor_tensor(
                                    dest[:, pb * KH:(pb + 1) * KH], ps[:],
                                    bb[:, c * KH:(c + 1) * KH], OP.add)
                            else:
                                e = (nc.scalar, nc.vector)[cnt[0] % 2] \
                                    if qb_ == 0 else nc.scalar
                                copy_on(e, dest[:, pb * KH:(pb + 1) * KH], ps[:])
                            cnt[0] += 1

                def conv1_part(parts):
                    for co, ck in parts:
                        i0, nr = CHUNKS4[ck]
                        nw = PW * nr
                        ps = psp.tile([P, 512], f32, name="ps", tag="ps")
                        for tap in range(9):
                            ty, tx = divmod(tap, 3)
                            base = PW * (i0 + ty) + tx
                            for ci in range(4):
                                off = ((co * 9 + tap) * 4 + ci) * P
                                nc.tensor.matmul(ps[:, 0:nw], w1c[:, off:off + P],
                                                 opad[ci][:, base:base + nw],
                                                 start=(tap == 0 and ci == 0),
                                                 stop=(tap == 8 and ci == 3))
                        copy_on(nc.vector if ck % 2 else nc.scalar,
                                y1[co][:, W * i0:W * (i0 + nr)].rearrange(
                                    "c (i j) -> c i j", j=W),
                                ps[:, 0:nw].rearrange("c (i j) -> c i j", j=PW)[:, :, 0:W])

                # ---- attention quarters (DVE/Act), interleaved with KQV ----
                def attn_quarter(hb):
                    CL = slice(hb * 1024, (hb + 1) * 1024)
                    GS = slice(hb * 16, hb * 16 + 16)
                    # scores
                    for t in range(R):
                        pr = prodp.tile([P, 2048], bf16, name="prod", tag="pr")
                        nc.vector.tensor_mul(pr[:, 0:1024], qbt[:, CL],
                                             kbt[t][:, CL])
                        nc.vector.tensor_reduce(
                            out=S3t[:, t, GS],
                            in_=pr[:, 0:1024].rearrange("p (g d) -> p g d", d=KD),
                            axis=AX.X, op=OP.add)
                    nc.vector.memset(S3t[:, R, GS], 0)  # zero-key slot
                    # softmax over T slots
                    nc.vector.tensor_reduce(out=mx[:, GS], in_=S3g[:, GS, 0:T],
                                            axis=AX.X, op=OP.max)
                    nc.vector.tensor_tensor(
                        ab3g[:, GS, 0:T], S3g[:, GS, 0:T],
                        mx[:, GS].unsqueeze(2).broadcast_to([P, 16, T]), OP.subtract)
                    nc.scalar.activation(ab3t[:, 0:T, GS], ab3t[:, 0:T, GS], ACTF.Exp)
                    nc.vector.tensor_reduce(out=zs[:, GS], in_=ab3g[:, GS, 0:T],
                                            axis=AX.X, op=OP.add)
                    if T > TOPK:
                        # sparse top-k on UNNORMALIZED e (exact algebra:
                        # w = max(e/Z - e4/Z - EPS, 0), out = w/(sum w + EPS)
                        # == max(e - e4 - Z*EPS, 0)/(sum(...) + Z*EPS)).
                        # 2nd-smallest of e via running (min, 2nd-min) chain.
                        nc.vector.tensor_tensor(mxp[:, GS], ab3t[:, 0, GS],
                                                ab3t[:, 1, GS], OP.min)
                        nc.vector.tensor_tensor(dmin[:, GS], ab3t[:, 0, GS],
                                                ab3t[:, 1, GS], OP.max)
                        for i in range(2, T):
                            nc.vector.tensor_tensor(mx[:, GS], mxp[:, GS],
                                                    ab3t[:, i, GS], OP.max)
                            nc.vector.tensor_tensor(dmin[:, GS], dmin[:, GS],
                                                    mx[:, GS], OP.min)
                            if i + 1 < T:
                                nc.vector.tensor_tensor(mxp[:, GS], mxp[:, GS],
                                                        ab3t[:, i, GS], OP.min)
                        # threshold = e_(4) + Z*EPS
                        nc.vector.scalar_tensor_tensor(
                            out=dmin[:, GS], in0=zs[:, GS], scalar=EPS,
                            in1=dmin[:, GS], op0=OP.mult, op1=OP.add)
                        nc.vector.tensor_tensor(
                            ab3g[:, GS, 0:T], ab3g[:, GS, 0:T],
                            dmin[:, GS].unsqueeze(2).broadcast_to([P, 16, T]),
                            OP.subtract)
                        nc.vector.tensor_scalar_max(ab3g[:, GS, 0:T],
                                                    ab3g[:, GS, 0:T], 0.0)
                        nc.vector.tensor_reduce(out=mx[:, GS],
                                                in_=ab3g[:, GS, 0:T],
                                                axis=AX.X, op=OP.add)
                        nc.vector.scalar_tensor_tensor(
                            out=mx[:, GS], in0=zs[:, GS], scalar=EPS,
                            in1=mx[:, GS], op0=OP.mult, op1=OP.add)
                        nc.vector.reciprocal(mx[:, GS], mx[:, GS])
                        nc.vector.tensor_tensor(
                            abb3g[:, GS, 0:T], ab3g[:, GS, 0:T],
                            mx[:, GS].unsqueeze(2).broadcast_to([P, 16, T]),
                            OP.mult)
                    else:
                        nc.vector.reciprocal(zs[:, GS], zs[:, GS])
                        nc.vector.tensor_tensor(
                            abb3g[:, GS, 0:T], ab3g[:, GS, 0:T],
                            zs[:, GS].unsqueeze(2).broadcast_to([P, 16, T]),
                            OP.mult)
                    # weighted sum -> obf half. v/obf are d-major
                    # (col = pb*512 + d*8 + h) so the attn broadcast is
                    # packed in the last dim -> DVE 2x mode.
                    for t in range(R):
                        abb = abb3t[:, t, GS].rearrange(
                            "p (b h) -> p b h", h=8).unsqueeze(2).broadcast_to(
                            [P, 2, KD, 8])
                        vv = vbt[t][:, CL].rearrange(
                            "p (b d h) -> p b d h", d=KD, h=8)
                        if t == 0:
                            nc.vector.tensor_tensor(
                                obf[:, CL].rearrange("p (b d h) -> p b d h",
                                                     d=KD, h=8),
                                vv, abb, OP.mult)
                        else:
                            tm = prodp.tile([P, 2048], bf16, name="wtm", tag="pr")
                            nc.vector.tensor_tensor(
                                tm[:, 0:1024].rearrange("p (b d h) -> p b d h",
                                                        d=KD, h=8),
                                vv, abb, OP.mult)
                            nc.vector.tensor_add(obf[:, CL], obf[:, CL],
                                                 tm[:, 0:1024])
                def transp_quarter(hb):
                    # PE transpose of this quarter into opad
                    for q in range(4):
                        tp = psp.tile([P, 512], bf16, name="tp", tag="ps")
                        for pbh in range(2):
                            pb = hb * 2 + pbh
                            nc.tensor.matmul(
                                tp[:, pbh * P:(pbh + 1) * P],
                                obf[:, pb * KH + q * P: pb * KH + (q + 1) * P],
                                ident[:], is_transpose=True, skip_group_check=True)
                        opv = opad[q][:, 0:PHW].rearrange("c (i j) -> c i j", j=PW)
                        copy_on(nc.scalar,
                                opv[:, 1 + 8 * hb:9 + 8 * hb, 1:W + 1],
                                tp[:, 0:256].rearrange("c (i j) -> c i j", j=W))

                # interleave: kqv(q) then attn(q) so DVE starts attention at
                # quarter granularity; transposes + conv1 mini-chunks trail.
                # conv1 mini-chunk k needs opad rows <= 8(k+1) = quarters
                # 0..k; co=0 runs ahead of co=1 so stats(0)+AR(0) issue early.
                # Per-layer split of co=1 chunks: late layers have a long DVE
                # attention chain, so embed more chunks to keep the PE fed;
                # early layers keep chunks back as AR(0) latency cover.
                embed = {0: [], 1: [], 2: [(1, 0)],
                         3: [(1, 0), (1, 1)]}[l]
                cover = [c for c in [(1, 0), (1, 1), (1, 2), (1, 3)]
                         if c not in embed]
                for q_ in range(4):
                    kqv_quarter(q_)
                    attn_quarter(q_)
                if l + 1 < layers:
                    load_kqv_weights(l + 1)
                for q_ in range(4):
                    # transposes first in the PE queue: conv1 chunks must
                    # never delay the transpose feeding the next chunk
                    transp_quarter(q_)
                    if q_ == 1:
                        conv1_part([(0, 0)])
                    elif q_ == 2:
                        conv1_part([(0, 1)] + embed[:1])
                    elif q_ == 3:
                        conv1_part([(0, 2)] + embed[1:])

                conv1_part([(0, 3)])

                # ---- stats + AllReduce per channel-half, pipelined ----
                def stats_send(i):
                    # sum on DVE in parallel with square+sum-of-squares as a
                    # SINGLE Act op (activation accum_out): the collective
                    # triggers ~1.2us earlier than the serial two-reduce path.
                    nc.vector.tensor_reduce(out=st[:, 2 * i:2 * i + 1],
                                            in_=y1[i][:], axis=AX.X, op=OP.add)
                    nc.scalar.activation(sqs[:], y1[i][:], ACTF.Square,
                                         accum_out=st[:, 2 * i + 1:2 * i + 2])
                    cci = dramp.tile([1, 2 * P], f32, name="cci")
                    cco = dramp.tile([1, 2 * P], f32, name="cco",
                                     addr_space="Shared")
                    nc.sync.dma_start(cci[0].rearrange("(p j) -> p j", j=2),
                                      st[:, 2 * i:2 * i + 2])
                    nc.gpsimd.collective_compute(
                        "AllReduce", OP.add,
                        replica_groups=[list(range(ncores))],
                        ins=[cci.opt()], outs=[cco.opt()])
                    nc.sync.dma_start(
                        gred[:, 2 * i:2 * i + 2],
                        cco[0].rearrange("(p j) -> p j", j=2))

                stats_send(0)
                conv1_part(cover)
                stats_send(1)
                for co in range(2):
                    nc.scalar.add(x[co][:], x[co][:], gob2[co])

                # ---- BN coef + h1 + conv2 (ci-split overlaps AG latency) ----
                NTOT = float(ncores * HW)

                def bn_h1(i):
                    nc.vector.tensor_scalar_mul(t1[i][:], gred[:, 2 * i:2 * i + 1],
                                                1.0 / NTOT)
                    nc.vector.tensor_scalar_mul(vart[i][:],
                                                gred[:, 2 * i + 1:2 * i + 2],
                                                1.0 / NTOT)
                    nc.vector.tensor_mul(sq[i][:], t1[i][:], t1[i][:])
                    nc.vector.tensor_sub(vart[i][:], vart[i][:], sq[i][:])
                    nc.vector.tensor_scalar_add(vart[i][:], vart[i][:], BN_EPS)
                    nc.scalar.activation(stdt[i][:], vart[i][:], ACTF.Sqrt)
                    nc.vector.reciprocal(stdt[i][:], stdt[i][:])
                    nc.vector.tensor_mul(Ac[i][:], bng[i], stdt[i][:])
                    nc.vector.tensor_mul(sq[i][:], t1[i][:], Ac[i][:])
                    nc.vector.tensor_sub(Bc[i][:], bnb[i], sq[i][:])
                    # split the relu so conv2's first row-chunk (reads h1
                    # rows <= 15) can start before the full h1 is written
                    h1v = h1p[i][:, 0:PHW].rearrange("c (i j) -> c i j", j=PW)
                    y1v = y1[i][:].rearrange("c (i j) -> c i j", j=W)
                    nc.scalar.activation(
                        h1v[:, 1:17, 1:W + 1], y1v[:, 0:16, :],
                        ACTF.Relu, bias=Bc[i][:], scale=Ac[i][:])
                    nc.scalar.activation(
                        h1v[:, 17:H + 1, 1:W + 1], y1v[:, 16:H, :],
                        ACTF.Relu, bias=Bc[i][:], scale=Ac[i][:])

                def conv2_taps(ps2, co, ci, start, stop):
                    for ck, (i0, nr) in enumerate(CHUNKS):
                        nw = PW * nr
                        for tap in range(9):
                            ty, tx = divmod(tap, 3)
                            base = PW * (i0 + ty) + tx
                            off = ((co * 9 + tap) * 2 + ci) * P
                            nc.tensor.matmul(
                                ps2[ck][:, 0:nw], w2s[:, off:off + P],
                                h1p[ci][:, base:base + nw],
                                start=(start and tap == 0),
                                stop=(stop and tap == 8))

                def resid(ps2, co):
                    for ck, (i0, nr) in enumerate(CHUNKS):
                        nw = PW * nr
                        xsl = x[co][:, W * i0:W * (i0 + nr)].rearrange(
                            "c (i j) -> c i j", j=W)
                        nc.vector.scalar_tensor_tensor(
                            out=xsl,
                            in0=ps2[ck][:, 0:nw].rearrange(
                                "c (i j) -> c i j", j=PW)[:, :, 0:W],
                            scalar=gam[co], in1=xsl, op0=OP.mult, op1=OP.add)
                        if l == layers - 1:
                            # final layer: stream the output per chunk so the
                            # last DMA overlaps the remaining resid work
                            nc.sync.dma_start(
                                out[co * P:(co + 1) * P,
                                    W * i0:W * (i0 + nr)],
                                x[co][:, W * i0:W * (i0 + nr)])
                    if l < layers - 1:
                        # split the bf16 copy: next layer's kqv quarter 0
                        # needs only the first 256 positions, so release
                        # that slice first
                        nc.scalar.copy(xb[co][:, 0:2 * P], x[co][:, 0:2 * P])
                        nc.scalar.copy(xb[co][:, 2 * P:], x[co][:, 2 * P:])

                # ci-split: both co groups' ci=0 taps hide AG(1) latency
                bn_h1(0)
                ps20 = [psp.tile([P, 512], f32, name="ps", tag="ps")
                        for _ in range(3)]
                ps21 = [psp.tile([P, 512], f32, name="ps", tag="ps")
                        for _ in range(3)]
                conv2_taps(ps20, 0, 0, True, False)
                conv2_taps(ps21, 1, 0, True, False)
                bn_h1(1)
                if l + 1 < layers:
                    # gate the 2.3MB w1 prefetch behind bn_h1(1): the dummy
                    # read adds a WAR dep so the transfer fires only after
                    # both BN collectives have completed, never inside them.
                    # (must read an INTERIOR h1p element — bn_h1 writes only
                    # rows/cols 1.., the [0,0] padding never changes)
                    nc.vector.tensor_tensor(wgate[:], w1s[(l + 1) % 2][0:1, 0:1],
                                            h1p[1][0:1, PW + 1:PW + 2], OP.add)
                    nc.sync.dma_start(w1s[(l + 1) % 2][:], w1d[l + 1])
                conv2_taps(ps20, 0, 1, False, True)
                resid(ps20, 0)
                conv2_taps(ps21, 1, 1, False, True)
                if l + 1 < layers:
                    nc.sync.dma_start(w2s[:], w2d[l + 1])
                resid(ps21, 1)
                if l + 1 < layers:
                    nc.sync.dma_start(cst[:], cstd[l + 1])

    nc.compile()
    return nc


def _host_prep(inputs):
    bf = ml_dtypes.bfloat16
    kw, kb = inputs["kw"], inputs["kb"]
    qw, qb = inputs["qw"], inputs["qb"]
    vw, vb = inputs["vw"], inputs["vb"]
    ow1, ow2 = inputs["ow1"], inputs["ow2"]
    ob2, gammas = inputs["ob2"], inputs["gammas"]

    def packkqv(w):  # [L, KH, C] -> [L, P, 2, KH]
        return w.reshape(L, KH, 2, P).transpose(0, 3, 2, 1)

    # v (and o) use d-major channel order: col d*8+h holds row h*64+d.
    # This makes the attention weighted-sum broadcast packed for DVE 2x.
    permdh = np.array([(c % 8) * 64 + c // 8 for c in range(KH)])
    d = {}
    wk3 = np.stack([packkqv(kw), packkqv(vw)[..., permdh],
                    packkqv(qw / 8.0)], axis=2)
    d["wkqvd"] = np.ascontiguousarray(wk3.reshape(L, P, 3 * 2 * KH)).astype(bf)
    bk3 = np.stack([kb, vb[:, permdh], qb / 8.0], axis=1)
    d["bkd"] = np.ascontiguousarray(bk3.reshape(L, 1, 3 * KH)).astype(bf)
    # conv1 input channels arrive via the PE transpose of d-major o:
    # opad[k] partition j holds channel c = (j%8)*64 + 16k + j//8.
    cinidx = np.array([[(j % 8) * 64 + 16 * k + j // 8 for j in range(P)]
                       for k in range(4)])
    a1 = ow1[:, :, cinidx]                      # [L, 256, 4, 128, 3, 3]
    a1 = a1.reshape(L, 2, P, 4, P, 3, 3).transpose(0, 4, 1, 5, 6, 3, 2)
    d["w1d"] = np.ascontiguousarray(a1.reshape(L, P, 72 * P)).astype(bf)
    a2 = ow2.reshape(L, 2, P, 2, P, 3, 3).transpose(0, 4, 1, 5, 6, 3, 2)
    d["w2d"] = np.ascontiguousarray(a2.reshape(L, P, 36 * P)).astype(bf)
    cstv = np.zeros((L, 2, P, 4), np.float32)
    cstv[..., 0] = inputs["bn_g"].reshape(L, 2, P)
    cstv[..., 1] = inputs["bn_b"].reshape(L, 2, P)
    cstv[..., 2] = (gammas[:, None] * ob2).reshape(L, 2, P)
    cstv[..., 3] = gammas[:, None, None]
    d["cstd"] = np.ascontiguousarray(
        cstv.transpose(0, 2, 1, 3).reshape(L, P, 8)).astype(np.float32)
    d["identd"] = np.eye(P, dtype=np.float32).astype(bf)
    return d


def kernel(**inputs):
    if "nc" not in _compiled:
        _compiled["nc"] = _build()
    nc = _compiled["nc"]
    shared = _host_prep(inputs)
    x = np.ascontiguousarray(inputs["x"].reshape(B, C, HW)).astype(np.float32)
    in_maps = []
    for c in range(NC):
        m = dict(shared)
        m["xin"] = x[c]
        in_maps.append(m)
    res = bass_utils.run_bass_kernel_spmd(nc, in_maps, core_ids=list(range(NC)))
    outs = np.stack([res.results[c]["out"] for c in range(NC)])
    return outs.reshape(B, C, H, W).astype(np.float32)

